# revision 32
# baseline (speedup 1.0000x reference)
"""Trainium2 Bass kernel for nn_MPNN (2-layer NNConv GNN + pooling + MLP).

Self-contained: host-side index planning (sort/pad/one-hot) + SPMD Bass
program on 8 NeuronCores. Edge-parallel sharding: edges sorted by dst node,
nodes split into 8 contiguous ranges balanced by edge count; each core
computes its node range's aggregates fully. Layer-1 node features are
computed replicated on every core from the full x (no collective); only the
layer-2 features are all-gathered, plus a final AllReduce of pooled sums.

Math: msg[e,o] = sum_{k,i} z[e,k] h[src_e,i] w2[k,i*64+o] + sum_i h[src_e,i] b2[i*64+o]
  u[e, k*64+i] = z[e,k]*h[e,i]              (outer product, DVE fp16 2x)
  G[n, j]      = segsum_e onehot * u        (one-hot scatter matmul, PE)
  agg[n, o]    = G @ W2v + H_agg @ B2r + h @ root + bias   (PE, DMA-transposed G)

Schedule: z for both layers precomputed upfront; gathers split in halves;
u-route pattern (ROUTE_PAT) balances the zrep copies across ACT/Pool so the
chunk-loop cadence matches the DVE outer-product rate.
"""

import sys

sys.path.insert(0, "/opt/trn_rl_repo")

import numpy as np
import ml_dtypes

import concourse.bass as bass
import concourse.bacc as bacc
import concourse.mybir as mybir
import concourse.tile as tile
from concourse.bass_utils import run_bass_kernel_spmd

BF16 = mybir.dt.float16  # 16-bit compute dtype (fp16: more mantissa than bf16)
FP32 = mybir.dt.float32
I16 = mybir.dt.int16
NB = np.float16
AF = mybir.ActivationFunctionType
OP = mybir.AluOpType

NCORES = 8
P = 128
HID = 64
NG = 64
NNODES = 10000
NEDGES = 64000
XF = 128
EDGE_CAP = 896          # max edges per node-chunk
TPC = EDGE_CAP // P     # e-tiles per chunk = 7
JBLK = 33               # j-blocks of 128: 4096 (k,i) + 64 (b2) + 64 pad
JEXT = JBLK * P         # 4224
NEG_SLOPE = 0.01


# ----------------------------------------------------------------------------
# Host planning
# ----------------------------------------------------------------------------
def _plan(edge_index, batch):
    src = np.asarray(edge_index[0], dtype=np.int64)
    dst = np.asarray(edge_index[1], dtype=np.int64)
    order = np.argsort(dst, kind="stable")
    src_s = src[order]
    dst_s = dst[order]

    deg = np.bincount(dst, minlength=NNODES).astype(np.int64)
    cume = np.concatenate([[0], np.cumsum(deg)])

    nb = [0]
    for c in range(1, NCORES):
        nb.append(int(np.searchsorted(cume, c * NEDGES / NCORES)))
    nb.append(NNODES)

    core_chunks = []
    for c in range(NCORES):
        chunks = []
        n0 = nb[c]
        while n0 < nb[c + 1]:
            n1 = n0
            e_acc = 0
            while n1 < nb[c + 1] and (n1 - n0) < P and e_acc + deg[n1] <= EDGE_CAP:
                e_acc += deg[n1]
                n1 += 1
            assert n1 > n0, f"node {n0} degree {deg[n0]} > EDGE_CAP"
            chunks.append((n0, n1))
            n0 = n1
        core_chunks.append(chunks)

    C = max(len(ch) for ch in core_chunks)
    ET = C * TPC
    SLOTS = ET * P
    NP_ = NCORES * C * P
    ZR = NP_
    assert NP_ + 16 < 32767

    ppos = np.full(NNODES, -1, dtype=np.int64)
    for c in range(NCORES):
        for ci, (a, b) in enumerate(core_chunks[c]):
            ppos[a:b] = c * C * P + ci * P + np.arange(b - a)

    per_core = []
    for c in range(NCORES):
        slot_src = np.full(SLOTS, ZR, dtype=np.int64)
        slot_edge = np.full(SLOTS, -1, dtype=np.int64)
        onehot = np.zeros((ET, P, P), dtype=NB)
        for ci, (a, b) in enumerate(core_chunks[c]):
            e0, e1 = int(cume[a]), int(cume[b])
            m = e1 - e0
            base = ci * TPC * P
            slot_src[base : base + m] = ppos[src_s[e0:e1]]
            slot_edge[base : base + m] = np.arange(e0, e1)
            loc = dst_s[e0:e1] - a
            sl = np.arange(m)
            onehot[base // P + sl // P, sl % P, loc] = 1.0

        idxw16 = slot_src.reshape(SLOTS // 16, 16).T.astype(np.int16)
        idxw = np.tile(idxw16, (8, 1))  # replicated per GPSIMD Q7 core

        batchoh = np.zeros((C, P, NG), dtype=NB)
        bat = np.asarray(batch, dtype=np.int64)
        for ci, (a, b) in enumerate(core_chunks[c]):
            batchoh[ci, np.arange(b - a), bat[a:b]] = 1.0

        per_core.append(
            dict(slot_edge=slot_edge, idxw=idxw, onehot=onehot, batchoh=batchoh,
                 chunks=core_chunks[c])
        )

    return dict(C=C, ET=ET, SLOTS=SLOTS, NP=NP_, per_core=per_core, ppos=ppos,
                order=order)


def _per_core_inputs(plan, c, edge_attr, x):
    C, ET, SLOTS = plan["C"], plan["ET"], plan["SLOTS"]
    pc = plan["per_core"][c]
    slot_edge = pc["slot_edge"]
    order = plan["order"]

    ea_s = np.asarray(edge_attr, dtype=np.float32)[order]
    eaT = np.zeros((ET, 33, P), dtype=NB)
    valid = slot_edge >= 0
    sl = np.arange(SLOTS)
    t_idx, j_idx = sl // P, sl % P
    eaT[t_idx[valid], :32, j_idx[valid]] = ea_s[slot_edge[valid]].astype(NB)
    eaT[t_idx[valid], 32, j_idx[valid]] = NB(1.0)

    xT = np.zeros((XF, C * P), dtype=NB)
    for ci, (a, b) in enumerate(pc["chunks"]):
        cols = ci * P + np.arange(b - a)
        xT[:, cols] = np.asarray(x[a:b], dtype=np.float32).T.astype(NB)
    return dict(eaT=eaT, xT=xT, idxw=pc["idxw"], onehot=pc["onehot"],
                batchoh=pc["batchoh"])


def _full_xT(plan, x):
    """Full x, transposed, in ppos (core-chunk-position) column order —
    identical for every core; lets each core compute h1 for ALL nodes
    locally, replacing the layer-1 AllGather."""
    NP_ = plan["NP"]
    ppos = plan["ppos"]
    xTf = np.zeros((XF, NP_), dtype=NB)
    xf = np.asarray(x, dtype=np.float32).T.astype(NB)  # [XF, NNODES]
    xTf[:, ppos] = xf
    return xTf


def _shared_inputs(inputs):
    f = lambda a: np.asarray(a, dtype=np.float32)
    sh = {}
    for l, pre in ((1, "efc1"), (2, "efc2")):
        w1 = f(inputs[f"{pre}_w1"])
        b1 = f(inputs[f"{pre}_b1"])
        sh[f"w1aug_{l}"] = np.concatenate([w1, b1[None, :]], 0).astype(NB)
        w2 = f(inputs[f"{pre}_w2"])
        b2 = f(inputs[f"{pre}_b2"])
        w2v = np.zeros((JEXT, HID), dtype=np.float32)
        w2v[: HID * HID] = w2.reshape(HID * HID, HID)  # j=k*64+i -> w2[k, i*64+o]
        w2v[HID * HID : HID * HID + HID] = b2.reshape(HID, HID)
        # SBUF layout [P, JBLK, HID]: partition = j within block
        sh[f"w2v_{l}"] = np.ascontiguousarray(
            w2v.reshape(JBLK, P, HID).transpose(1, 0, 2)
        ).astype(NB)
        g = "gc1" if l == 1 else "gc2"
        sh[f"root_{l}"] = f(inputs[f"{g}_root"]).astype(NB)
        sh[f"rootb_{l}"] = f(inputs[f"{g}_bias"])[None, :].astype(NB)
    sh["nfc_w"] = f(inputs["nfc_w"]).astype(NB)
    sh["nfc_b"] = f(inputs["nfc_b"])[None, :].astype(NB)
    sh["fc1_w"] = f(inputs["fc1_w"]).astype(NB)
    sh["fc1_b"] = f(inputs["fc1_b"])[None, :].astype(NB)
    sh["fc2_w"] = f(inputs["fc2_w"]).astype(NB)
    sh["fc2_b"] = f(inputs["fc2_b"])[None, :].astype(NB)
    sh["identity"] = np.eye(P, dtype=NB)
    sh["ones_row"] = np.ones((1, 512), dtype=NB)
    sh["zrow"] = np.zeros((16, P), dtype=NB)
    return sh


# ----------------------------------------------------------------------------
# Bass program
# ----------------------------------------------------------------------------
# per-tile u-route pattern, cycled per chunk:
#  A: zrep on ACT + DVE 2x TT;  G: zrep on GPSIMD + DVE 2x TT
#  D: direct 1x TT on DVE;      P: direct multiply on GPSIMD
import os as _os
ROUTE_PAT = [list(r) for r in _os.environ.get("ROUTE_PAT", "AGAAGAG,AGAGAGA").split(",")]


def _leaky(nc, sm, out_sb, in_ps, tag):
    """out_sb = leaky_relu(in_ps) via 2 DVE ops (Lrelu unsupported in sim)."""
    pp, ff = out_sb.shape[0], out_sb.shape[-1]
    t1 = sm.tile([pp, ff], BF16, tag=f"lk_{tag}")
    nc.vector.tensor_scalar_mul(t1[:], in_ps, NEG_SLOPE)
    nc.vector.tensor_tensor(out=out_sb, in0=in_ps, in1=t1[:], op=OP.max)


def _build(C, phases=99, collectives=True):
    ET = C * TPC
    SLOTS = ET * P
    NP_ = NCORES * C * P
    NPH = NP_ + 16
    NCOLS = C * P
    G512 = (NCOLS + 511) // 512

    ndev = NCORES if collectives else 1
    nc = bacc.Bacc(
        "TRN2", target_bir_lowering=False, debug=False, num_devices=ndev
    )
    core_ids = list(range(NCORES))

    def EIN(name, shape, dt):
        return nc.dram_tensor(name, list(shape), dt, kind="ExternalInput")

    eaT = EIN("eaT", (ET, 33, P), BF16)
    onehot = EIN("onehot", (ET, P, P), BF16)
    idxw = EIN("idxw", (P, SLOTS // 16), I16)
    xT = EIN("xT", (XF, NCOLS), BF16)
    xTf = EIN("xTf", (XF, NP_), BF16)
    batchoh = EIN("batchoh", (C, P, NG), BF16)
    w1aug = [EIN(f"w1aug_{l}", (33, HID), BF16) for l in (1, 2)]
    w2v = [EIN(f"w2v_{l}", (P, JBLK, HID), BF16) for l in (1, 2)]
    root = [EIN(f"root_{l}", (HID, HID), BF16) for l in (1, 2)]
    rootb = [EIN(f"rootb_{l}", (1, HID), BF16) for l in (1, 2)]
    nfc_w = EIN("nfc_w", (XF, HID), BF16)
    nfc_b = EIN("nfc_b", (1, HID), BF16)
    fc1_w = EIN("fc1_w", (HID, 32), BF16)
    fc1_b = EIN("fc1_b", (1, 32), BF16)
    fc2_w = EIN("fc2_w", (32, 1), BF16)
    fc2_b = EIN("fc2_b", (1, 1), BF16)
    identity = EIN("identity", (P, P), BF16)
    ones_row = EIN("ones_row", (1, 512), BF16)
    zrow = EIN("zrow", (16, P), BF16)

    outv = nc.dram_tensor("outv", [1, NG], FP32, kind="ExternalOutput")

    hbuf = [nc.dram_tensor(f"hbuf{l}", [NPH, P], BF16, addr_space="Shared")
            for l in (1, 2)]
    hown = nc.dram_tensor("hown2", [NCOLS, P], BF16)
    gdram = nc.dram_tensor("gdram", [NCOLS, JEXT], BF16)
    ccin = nc.dram_tensor("ccin", [HID, NG], FP32)
    ccout = nc.dram_tensor("ccout", [HID, NG], FP32, addr_space="Shared")

    with tile.TileContext(nc) as tc:
        with (
            tc.tile_pool(name="const", bufs=1) as cpool,
            tc.tile_pool(name="resid", bufs=1) as rpool,
            tc.tile_pool(name="stream", bufs=2) as spool,
            tc.tile_pool(name="small", bufs=4) as smpool,
            tc.tile_pool(name="ps", bufs=8, space="PSUM") as ps,
        ):
            _lc = [0]
            def load_const(t, dt=BF16):
                sb = cpool.tile(list(t.shape), dt, tag=t.name)
                eng = nc.sync if _lc[0] % 2 == 0 else nc.scalar
                _lc[0] += 1
                eng.dma_start(out=sb[:], in_=t[:])
                return sb

            sb_w1 = [load_const(w1aug[i]) for i in range(2)]
            sb_w2v = [load_const(w2v[i]) for i in range(2)]
            sb_root = [load_const(root[i]) for i in range(2)]
            sb_rootb = [load_const(rootb[i]) for i in range(2)]
            sb_nfc_w = load_const(nfc_w)
            sb_nfc_b = load_const(nfc_b)
            sb_fc1_w = load_const(fc1_w)
            sb_fc1_b = load_const(fc1_b)
            sb_fc2_w = load_const(fc2_w)
            sb_fc2_b = load_const(fc2_b)
            sb_ident = load_const(identity)
            sb_ones = load_const(ones_row)
            sb_idx = load_const(idxw, dt=I16)


            def _dummy_out():
                osb0 = smpool.tile([1, NG], FP32, tag="osb0")
                nc.gpsimd.memset(osb0[:], 0.0)
                nc.sync.dma_start(out=outv[:], in_=osb0[:])

            zr_sb = cpool.tile([16, P], BF16, tag="zrsb")
            nc.sync.dma_start(out=zr_sb[:], in_=zrow[:])
            for l in range(2):
                nc.sync.dma_start(out=hbuf[l][NP_:NPH, :], in_=zr_sb[:])

            # ---- startup: h1 (own + all-nodes) and z for both layers,
            # round-robin interleaved so PSUM slots and engines overlap ----
            h1T = rpool.tile([HID, NCOLS], BF16, tag="h1T")
            NPB = NP_ // 512
            XGRP = 4  # 512-blocks per xTf DMA
            xtf_box = [None]

            def emit_h1own(g):
                c0, c1 = g * 512, min((g + 1) * 512, NCOLS)
                w = c1 - c0
                xt = smpool.tile([XF, 512], BF16, tag="xt", bufs=2)
                nc.sync.dma_start(out=xt[:, :w], in_=xT[:, c0:c1])
                pt = ps.tile([P, 512], FP32, tag="b512")
                agg = pt[:HID, :w]
                nc.tensor.matmul(agg, sb_nfc_w[:], xt[:, :w], start=True, stop=False)
                nc.tensor.matmul(agg, sb_nfc_b[:], sb_ones[:1, :w],
                                 start=False, stop=True)
                _leaky(nc, smpool, h1T[:, c0:c1], agg, "h1")

            def emit_h1full(g):
                c0 = g * 512
                if g % XGRP == 0:
                    gw = min(XGRP * 512, NP_ - c0)
                    xtf_box[0] = smpool.tile([XF, XGRP * 512], BF16, tag="xtf",
                                             bufs=2, name="xtf")
                    nc.scalar.dma_start(
                        out=xtf_box[0][:, :gw], in_=xTf[:, c0 : c0 + gw])
                xt = xtf_box[0][:, (g % XGRP) * 512 : (g % XGRP + 1) * 512]
                pt = ps.tile([P, 512], FP32, tag="b512")
                agg = pt[:HID, :]
                nc.tensor.matmul(agg, sb_nfc_w[:], xt, start=True, stop=False)
                nc.tensor.matmul(agg, sb_nfc_b[:], sb_ones[:1, :512],
                                 start=False, stop=True)
                htmp = smpool.tile([HID, 512], BF16, tag="hful", bufs=3)
                _leaky(nc, smpool, htmp[:], agg, "hf")
                tp = ps.tile([P, 1024], BF16, tag="b512")
                for q in range(4):
                    nc.tensor.transpose(
                        tp[:, q * HID : (q + 1) * HID],
                        htmp[:, q * P : (q + 1) * P], sb_ident[:HID, :HID],
                    )
                rows = smpool.tile([P, 4 * HID], BF16, tag="rows4", bufs=3)
                nc.scalar.activation(rows[:], tp[:, : 4 * HID], AF.Copy)
                nc.sync.dma_start(
                    out=hbuf[0][c0 : c0 + 512, :HID].rearrange(
                        "(q p) h -> p q h", q=4
                    ),
                    in_=rows[:].rearrange("p (q h) -> p q h", q=4),
                )

            zsb_l = []
            EPARTS = 2 if C <= 10 else 4
            EH = (ET + EPARTS - 1) // EPARTS
            et_tiles = {}

            def emit_zgroup(l, half, tg):
                zsb = zsb_l[l]
                h0, h1 = half * EH, min((half + 1) * EH, ET)  # half = part idx
                if (l, half) not in et_tiles:
                    et = smpool.tile([33, EH, P], BF16, tag="eat", bufs=2)
                    nc.sync.dma_start(
                        out=et[:, : h1 - h0, :],
                        in_=eaT[h0:h1, :, :].rearrange("t f j -> f t j"),
                    )
                    et_tiles[(l, half)] = et
                et = et_tiles[(l, half)]
                t0, t1 = h0 + tg * 8, min(h0 + tg * 8 + 8, h1)
                zp = ps.tile([P, 512], FP32, tag="b512")
                for t in range(t0, t1):
                    nc.tensor.matmul(
                        zp[:, (t - t0) * HID : (t - t0 + 1) * HID],
                        et[:, t - h0, :], sb_w1[l][:], start=True, stop=True,
                    )
                nc.scalar.activation(
                    zsb[:, t0:t1, :].rearrange("p a b -> p (a b)"),
                    zp[:, : (t1 - t0) * HID], AF.Relu,
                )

            if phases > 0:
                zsb_l = [rpool.tile([P, ET, HID], BF16, tag=f"z{l}",
                                    name=f"zsb{l}")
                         for l in range(2)]
                zjobs = [(0, half, tg)
                         for half in range(EPARTS)
                         for tg in range((min((half + 1) * EH, ET)
                                          - half * EH + 7) // 8)]
                zjobs_l1 = [(1, half, tg)
                            for half in range(EPARTS)
                            for tg in range((min((half + 1) * EH, ET)
                                             - half * EH + 7) // 8)]
            else:
                zjobs = []
                zjobs_l1 = []
            zi = 0
            for g in range(NPB):
                emit_h1full(g)
                if zi < len(zjobs):
                    emit_zgroup(*zjobs[zi])
                    zi += 1
            while zi < len(zjobs):
                emit_zgroup(*zjobs[zi])
                zi += 1
            for g in range(G512):
                emit_h1own(g)

            if phases <= 0:
                _dummy_out()
            hprevT = h1T
            for l in range(2) if phases > 0 else []:
                zsb = zsb_l[l]
                if phases <= 1 + 10 * l:
                    _dummy_out()
                    break
                # ---- gather h rows (split for pipelining) ----
                hs = rpool.tile([P, ET, P], BF16, tag="hs",
                                bufs=2 if C <= 10 else 1)
                for t0s, t1s in ((0, ET // 2), (ET // 2, ET)):
                    ntile = t1s - t0s
                    nc.gpsimd.dma_gather(
                        out_ap=hs[:, t0s:t1s, :], in_ap=hbuf[l][:, :],
                        idxs_ap=sb_idx[:, t0s * 8 : t1s * 8],
                        num_idxs=ntile * P, num_idxs_reg=ntile * P,
                        elem_size=P, single_packet=False,
                    )

                if phases <= 2 + 10 * l:
                    _dummy_out()
                    break

                hnextT = rpool.tile([HID, NCOLS], BF16, tag=f"hn_{l}")


                # ---- per chunk: u outer products + scatter matmuls ----
                for ci in range(C):
                    gq = [ps.tile([P, 512], FP32, tag="b512", name=f"gq{_q}")
                          for _q in range(8)]
                    oh = smpool.tile([P, TPC, P], BF16, tag="oh", bufs=3)
                    nc.sync.dma_start(
                        out=oh[:],
                        in_=onehot[ci * TPC : (ci + 1) * TPC, :, :].rearrange(
                            "t p j -> p t j"
                        ),
                    )
                    for tt in range(TPC):
                        t = ci * TPC + tt
                        zin = zsb[:, t, :, None].broadcast_to([P, HID, HID])
                        hin = hs[:, t, None, :HID].broadcast_to([P, HID, HID])
                        u = spool.tile([P, HID * HID], BF16, tag="u", bufs=3)
                        uview = u[:].rearrange("p (a b) -> p a b", a=HID)
                        route = ROUTE_PAT[ci % len(ROUTE_PAT)][tt]
                        if route in ("A", "G"):
                            zrep = spool.tile([P, HID * HID], BF16, tag="zrep", bufs=3)
                            zview = zrep[:].rearrange("p (a b) -> p a b", a=HID)
                            if route == "A":
                                nc.scalar.activation(zview, zin, AF.Copy)
                            else:
                                nc.gpsimd.tensor_copy(zview, zin)
                            nc.vector.tensor_tensor(
                                out=uview, in0=zview, in1=hin, op=OP.mult
                            )
                        elif route == "D":
                            # direct 1x-mode TT on DVE (broadcast APs)
                            nc.vector.tensor_tensor(
                                out=uview, in0=zin, in1=hin, op=OP.mult
                            )
                        else:  # "P": direct multiply on GPSIMD
                            nc.gpsimd.scalar_tensor_tensor(
                                out=uview, in0=zin, scalar=1.0, in1=hin,
                                op0=OP.mult, op1=OP.mult,
                            )
                        for q in range(8):
                            nc.tensor.matmul(
                                gq[q][:], oh[:, tt, :], u[:, q * 512 : (q + 1) * 512],
                                start=(tt == 0), stop=(tt == TPC - 1),
                            )
                    # b2 pass: H_agg chunk = segsum of h_src rows
                    gb = ps.tile([P, 512], FP32, tag="b512")
                    for t2 in range(TPC):
                        nc.tensor.matmul(
                            gb[:, :HID], oh[:, t2, :], hs[:, ci * TPC + t2, :HID],
                            start=(t2 == 0), stop=(t2 == TPC - 1),
                        )
                    gs = smpool.tile([P, JEXT], BF16, tag="gs", bufs=2)
                    for q in range(8):
                        dst = gs[:, q * 512 : (q + 1) * 512]
                        if q % 2 == 0:
                            nc.scalar.activation(dst, gq[q][:], AF.Copy)
                        else:
                            nc.vector.tensor_copy(dst, gq[q][:])
                    nc.scalar.activation(gs[:, 4096 : 4096 + HID], gb[:, :HID], AF.Copy)
                    nc.gpsimd.memset(gs[:, 4096 + HID : JEXT], 0.0)
                    nc.sync.dma_start(
                        out=gdram[ci * P : (ci + 1) * P, :], in_=gs[:]
                    )

                if phases <= 3 + 10 * l:
                    _dummy_out()
                    break
                if phases <= 4 + 10 * l:
                    _dummy_out()
                    break
                # ---- G^T via DMA transpose, streamed per j-block ----
                aggs = [ps.tile([P, 512], FP32, tag="b512", name=f"agg{l}_{_g}")
                        for _g in range(G512)]
                for b in range(JBLK):
                    gtb = smpool.tile([P, NCOLS], BF16, tag="gtb", bufs=4)
                    nc.sync.dma_start_transpose(
                        gtb[:], gdram[:, b * P : (b + 1) * P]
                    )
                    for g in range(G512):
                        c0, c1 = g * 512, min((g + 1) * 512, NCOLS)
                        nc.tensor.matmul(
                            aggs[g][:HID, : c1 - c0], sb_w2v[l][:, b, :],
                            gtb[:, c0:c1], start=(b == 0), stop=False,
                        )
                    if l == 0 and b % 3 == 0 and zjobs_l1:
                        emit_zgroup(*zjobs_l1.pop(0))
                while l == 0 and zjobs_l1:
                    emit_zgroup(*zjobs_l1.pop(0))
                for g in range(G512):
                    c0, c1 = g * 512, min((g + 1) * 512, NCOLS)
                    w = c1 - c0
                    agg = aggs[g][:HID, :w]
                    nc.tensor.matmul(agg, sb_root[l][:], hprevT[:, c0:c1],
                                     start=False, stop=False)
                    nc.tensor.matmul(agg, sb_rootb[l][:], sb_ones[:1, :w],
                                     start=False, stop=True)
                    _leaky(nc, smpool, hnextT[:, c0:c1], agg, "hn")
                    if l == 0:
                        for ci2 in range(c0 // P, (c1 + P - 1) // P):
                            tp = ps.tile([P, 1024], BF16, tag="b512")
                            nc.tensor.transpose(
                                tp[:, :HID], hnextT[:, ci2 * P : (ci2 + 1) * P],
                                sb_ident[:HID, :HID],
                            )
                            rows = smpool.tile([P, HID], BF16, tag="rows")
                            nc.scalar.activation(rows[:], tp[:, :HID], AF.Copy)
                            nc.sync.dma_start(
                                out=hown[ci2 * P : (ci2 + 1) * P, :HID],
                                in_=rows[:],
                            )

                if phases <= 5 + 10 * l:
                    _dummy_out()
                    break
                # ---- tail ----
                if l == 0:
                    if collectives:
                        nc.gpsimd.collective_compute(
                            "AllGather", OP.bypass, replica_groups=[core_ids],
                            ins=[hown[:]], outs=[hbuf[1][0:NP_, :]],
                        )
                    else:
                        nc.sync.dma_start(out=hbuf[1][0:NCOLS, :], in_=hown[:])
                    hprevT = hnextT
                else:
                    # pooling: hg^T[f, g] = sum_n hx2[n, f] batchoh[n, g]
                    hg_ps = ps.tile([P, 512], FP32, tag="b512")
                    hg = hg_ps[:HID, :NG]
                    boall = smpool.tile([P, C, NG], BF16, tag="bo")
                    nc.sync.dma_start(
                        out=boall[:],
                        in_=batchoh[:, :, :].rearrange("c p g -> p c g"),
                    )
                    for ci in range(C):
                        tp = ps.tile([P, 1024], BF16, tag="b512")
                        nc.tensor.transpose(
                            tp[:, :HID], hnextT[:, ci * P : (ci + 1) * P],
                            sb_ident[:HID, :HID],
                        )
                        rows = smpool.tile([P, HID], BF16, tag="rows")
                        nc.scalar.activation(rows[:], tp[:, :HID], AF.Copy)
                        nc.tensor.matmul(hg, rows[:], boall[:, ci, :],
                                         start=(ci == 0), stop=(ci == C - 1))
                    hg_sb = smpool.tile([HID, NG], FP32, tag="hgsb")
                    nc.vector.tensor_copy(hg_sb[:], hg)
                    nc.sync.dma_start(out=ccin[:], in_=hg_sb[:])
                    if collectives:
                        nc.gpsimd.collective_compute(
                            "AllReduce", OP.add, replica_groups=[core_ids],
                            ins=[ccin[:]], outs=[ccout[:]],
                        )
                    else:
                        nc.sync.dma_start(out=ccout[:], in_=ccin[:])
                    hgT = smpool.tile([HID, NG], FP32, tag="hgT")
                    nc.sync.dma_start(out=hgT[:], in_=ccout[:])
                    hgTb = smpool.tile([HID, NG], BF16, tag="hgTb")
                    nc.scalar.activation(hgTb[:], hgT[:], AF.Copy)
                    z1p = ps.tile([P, 512], FP32, tag="b512")
                    z1 = z1p[:32, :NG]
                    nc.tensor.matmul(z1, sb_fc1_w[:], hgTb[:], start=True, stop=False)
                    nc.tensor.matmul(z1, sb_fc1_b[:], sb_ones[:1, :NG],
                                     start=False, stop=True)
                    z1sb = smpool.tile([32, NG], BF16, tag="z1sb")
                    _leaky(nc, smpool, z1sb[:], z1, "z1")
                    op_ = ps.tile([P, 512], FP32, tag="b512")
                    o1 = op_[:1, :NG]
                    nc.tensor.matmul(o1, sb_fc2_w[:], z1sb[:], start=True, stop=False)
                    nc.tensor.matmul(o1, sb_fc2_b[:], sb_ones[:1, :NG],
                                     start=False, stop=True)
                    osb = smpool.tile([1, NG], FP32, tag="osb")
                    nc.vector.tensor_copy(osb[:], o1)
                    nc.sync.dma_start(out=outv[:], in_=osb[:])

    nc.compile()
    return nc


# ----------------------------------------------------------------------------
# Entry point
# ----------------------------------------------------------------------------
_BUILD_CACHE = {}


def build_in_maps(inputs):
    edge_index = np.asarray(inputs["edge_index"])
    batch = np.asarray(inputs["batch"])
    plan = _plan(edge_index, batch)
    C = plan["C"]
    shared = _shared_inputs(inputs)
    shared["xTf"] = _full_xT(plan, inputs["x"])
    in_maps = []
    for c in range(NCORES):
        m = dict(shared)
        m.update(_per_core_inputs(plan, c, inputs["edge_attr"], inputs["x"]))
        in_maps.append({k: np.ascontiguousarray(v) for k, v in m.items()})
    return C, in_maps


def kernel(**inputs) -> np.ndarray:
    C, in_maps = build_in_maps(inputs)
    if C not in _BUILD_CACHE:
        _BUILD_CACHE[C] = _build(C)
    nc = _BUILD_CACHE[C]
    res = run_bass_kernel_spmd(nc, in_maps, list(range(NCORES)))
    out = np.asarray(res.results[0]["outv"], dtype=np.float32)
    return out.reshape(NG, 1)


if __name__ == "__main__":
    import reference

    inp = {k: np.asarray(v) for k, v in reference.setup_inputs().items()}
    got = kernel(**inp)
    exp = np.asarray(reference.reference(**inp))
    num = np.linalg.norm(got - exp)
    den = np.linalg.norm(exp) + 1e-30
    print("rel l2 error:", num / den)
    print("max abs diff:", np.abs(got - exp).max(), "absmax:", np.abs(exp).max())



# revision 33
# speedup vs baseline: 1.0325x; 1.0325x over previous
"""Trainium2 Bass kernel for nn_MPNN (2-layer NNConv GNN + pooling + MLP).

Self-contained: host-side index planning (sort/pad/one-hot) + SPMD Bass
program on 8 NeuronCores. Edge-parallel sharding: edges sorted by dst node,
nodes split into 8 contiguous ranges balanced by edge count; each core
computes its node range's aggregates fully. Layer-1 node features are
computed replicated on every core from the full x (no collective); only the
layer-2 features are all-gathered, plus a final AllReduce of pooled sums.

Math: msg[e,o] = sum_{k,i} z[e,k] h[src_e,i] w2[k,i*64+o] + sum_i h[src_e,i] b2[i*64+o]
  u[e, k*64+i] = z[e,k]*h[e,i]              (outer product, DVE fp16 2x)
  G[n, j]      = segsum_e onehot * u        (one-hot scatter matmul, PE)
  agg[n, o]    = G @ W2v + H_agg @ B2r + h @ root + bias   (PE, DMA-transposed G)

Schedule: z for both layers precomputed upfront; gathers split in halves;
u-route pattern (ROUTE_PAT) balances the zrep copies across ACT/Pool so the
chunk-loop cadence matches the DVE outer-product rate.
"""

import sys

sys.path.insert(0, "/opt/trn_rl_repo")

import numpy as np
import ml_dtypes

import concourse.bass as bass
import concourse.bacc as bacc
import concourse.mybir as mybir
import concourse.tile as tile
from concourse.bass_utils import run_bass_kernel_spmd

BF16 = mybir.dt.float16  # 16-bit compute dtype (fp16: more mantissa than bf16)
FP32 = mybir.dt.float32
I16 = mybir.dt.int16
NB = np.float16
AF = mybir.ActivationFunctionType
OP = mybir.AluOpType

NCORES = 8
P = 128
HID = 64
NG = 64
NNODES = 10000
NEDGES = 64000
XF = 128
EDGE_CAP = 896          # max edges per node-chunk
TPC = EDGE_CAP // P     # e-tiles per chunk = 7
JBLK = 33               # j-blocks of 128: 4096 (k,i) + 64 (b2) + 64 pad
JEXT = JBLK * P         # 4224
NEG_SLOPE = 0.01


# ----------------------------------------------------------------------------
# Host planning
# ----------------------------------------------------------------------------
def _plan(edge_index, batch):
    src = np.asarray(edge_index[0], dtype=np.int64)
    dst = np.asarray(edge_index[1], dtype=np.int64)
    order = np.argsort(dst, kind="stable")
    src_s = src[order]
    dst_s = dst[order]

    deg = np.bincount(dst, minlength=NNODES).astype(np.int64)
    cume = np.concatenate([[0], np.cumsum(deg)])

    nb = [0]
    for c in range(1, NCORES):
        nb.append(int(np.searchsorted(cume, c * NEDGES / NCORES)))
    nb.append(NNODES)

    core_chunks = []
    for c in range(NCORES):
        chunks = []
        n0 = nb[c]
        while n0 < nb[c + 1]:
            n1 = n0
            e_acc = 0
            while n1 < nb[c + 1] and (n1 - n0) < P and e_acc + deg[n1] <= EDGE_CAP:
                e_acc += deg[n1]
                n1 += 1
            assert n1 > n0, f"node {n0} degree {deg[n0]} > EDGE_CAP"
            chunks.append((n0, n1))
            n0 = n1
        core_chunks.append(chunks)

    C = max(len(ch) for ch in core_chunks)
    ET = C * TPC
    SLOTS = ET * P
    NP_ = NCORES * C * P
    ZR = NP_
    assert NP_ + 16 < 32767

    ppos = np.full(NNODES, -1, dtype=np.int64)
    for c in range(NCORES):
        for ci, (a, b) in enumerate(core_chunks[c]):
            ppos[a:b] = c * C * P + ci * P + np.arange(b - a)

    per_core = []
    for c in range(NCORES):
        slot_src = np.full(SLOTS, ZR, dtype=np.int64)
        slot_edge = np.full(SLOTS, -1, dtype=np.int64)
        onehot = np.zeros((ET, P, P), dtype=NB)
        for ci, (a, b) in enumerate(core_chunks[c]):
            e0, e1 = int(cume[a]), int(cume[b])
            m = e1 - e0
            base = ci * TPC * P
            slot_src[base : base + m] = ppos[src_s[e0:e1]]
            slot_edge[base : base + m] = np.arange(e0, e1)
            loc = dst_s[e0:e1] - a
            sl = np.arange(m)
            onehot[base // P + sl // P, sl % P, loc] = 1.0

        idxw16 = slot_src.reshape(SLOTS // 16, 16).T.astype(np.int16)
        idxw = np.tile(idxw16, (8, 1))  # replicated per GPSIMD Q7 core

        batchoh = np.zeros((C, P, NG), dtype=NB)
        bat = np.asarray(batch, dtype=np.int64)
        for ci, (a, b) in enumerate(core_chunks[c]):
            batchoh[ci, np.arange(b - a), bat[a:b]] = 1.0

        per_core.append(
            dict(slot_edge=slot_edge, idxw=idxw, onehot=onehot, batchoh=batchoh,
                 chunks=core_chunks[c])
        )

    return dict(C=C, ET=ET, SLOTS=SLOTS, NP=NP_, per_core=per_core, ppos=ppos,
                order=order)


def _per_core_inputs(plan, c, edge_attr, x):
    C, ET, SLOTS = plan["C"], plan["ET"], plan["SLOTS"]
    pc = plan["per_core"][c]
    slot_edge = pc["slot_edge"]
    order = plan["order"]

    ea_s = np.asarray(edge_attr, dtype=np.float32)[order]
    eaT = np.zeros((ET, 33, P), dtype=NB)
    valid = slot_edge >= 0
    sl = np.arange(SLOTS)
    t_idx, j_idx = sl // P, sl % P
    eaT[t_idx[valid], :32, j_idx[valid]] = ea_s[slot_edge[valid]].astype(NB)
    eaT[t_idx[valid], 32, j_idx[valid]] = NB(1.0)

    xT = np.zeros((XF, C * P), dtype=NB)
    for ci, (a, b) in enumerate(pc["chunks"]):
        cols = ci * P + np.arange(b - a)
        xT[:, cols] = np.asarray(x[a:b], dtype=np.float32).T.astype(NB)
    return dict(eaT=eaT, xT=xT, idxw=pc["idxw"], onehot=pc["onehot"],
                batchoh=pc["batchoh"])


def _full_xT(plan, x):
    """Full x, transposed, in ppos (core-chunk-position) column order —
    identical for every core; lets each core compute h1 for ALL nodes
    locally, replacing the layer-1 AllGather."""
    NP_ = plan["NP"]
    ppos = plan["ppos"]
    xTf = np.zeros((XF, NP_), dtype=NB)
    xf = np.asarray(x, dtype=np.float32).T.astype(NB)  # [XF, NNODES]
    xTf[:, ppos] = xf
    return xTf


def _shared_inputs(inputs):
    f = lambda a: np.asarray(a, dtype=np.float32)
    sh = {}
    for l, pre in ((1, "efc1"), (2, "efc2")):
        w1 = f(inputs[f"{pre}_w1"])
        b1 = f(inputs[f"{pre}_b1"])
        sh[f"w1aug_{l}"] = np.concatenate([w1, b1[None, :]], 0).astype(NB)
        w2 = f(inputs[f"{pre}_w2"])
        b2 = f(inputs[f"{pre}_b2"])
        w2v = np.zeros((JEXT, HID), dtype=np.float32)
        w2v[: HID * HID] = w2.reshape(HID * HID, HID)  # j=k*64+i -> w2[k, i*64+o]
        w2v[HID * HID : HID * HID + HID] = b2.reshape(HID, HID)
        # SBUF layout [P, JBLK, HID]: partition = j within block
        sh[f"w2v_{l}"] = np.ascontiguousarray(
            w2v.reshape(JBLK, P, HID).transpose(1, 0, 2)
        ).astype(NB)
        g = "gc1" if l == 1 else "gc2"
        sh[f"root_{l}"] = f(inputs[f"{g}_root"]).astype(NB)
        sh[f"rootb_{l}"] = f(inputs[f"{g}_bias"])[None, :].astype(NB)
    sh["nfc_w"] = f(inputs["nfc_w"]).astype(NB)
    sh["nfc_b"] = f(inputs["nfc_b"])[None, :].astype(NB)
    sh["fc1_w"] = f(inputs["fc1_w"]).astype(NB)
    sh["fc1_b"] = f(inputs["fc1_b"])[None, :].astype(NB)
    sh["fc2_w"] = f(inputs["fc2_w"]).astype(NB)
    sh["fc2_b"] = f(inputs["fc2_b"])[None, :].astype(NB)
    sh["identity"] = np.eye(P, dtype=NB)
    sh["ones_row"] = np.ones((1, 512), dtype=NB)
    sh["zrow"] = np.zeros((16, P), dtype=NB)
    return sh


# ----------------------------------------------------------------------------
# Bass program
# ----------------------------------------------------------------------------
# per-tile u-route pattern, cycled per chunk:
#  A: zrep on ACT + DVE 2x TT;  G: zrep on GPSIMD + DVE 2x TT
#  D: direct 1x TT on DVE;      P: direct multiply on GPSIMD
import os as _os
ROUTE_PAT = [list(r) for r in _os.environ.get("ROUTE_PAT", "AGAAGAG,AGAAGAG,AGAADAG,AGAAGAG,AGAAGAG,AGAADAG,AGAAGAG,AGAAGAG,AGAADAG,AGAAGAG").split(",")]


def _leaky(nc, sm, out_sb, in_ps, tag):
    """out_sb = leaky_relu(in_ps) via 2 DVE ops (Lrelu unsupported in sim)."""
    pp, ff = out_sb.shape[0], out_sb.shape[-1]
    t1 = sm.tile([pp, ff], BF16, tag=f"lk_{tag}")
    nc.vector.tensor_scalar_mul(t1[:], in_ps, NEG_SLOPE)
    nc.vector.tensor_tensor(out=out_sb, in0=in_ps, in1=t1[:], op=OP.max)


def _build(C, phases=99, collectives=True):
    ET = C * TPC
    SLOTS = ET * P
    NP_ = NCORES * C * P
    NPH = NP_ + 16
    NCOLS = C * P
    G512 = (NCOLS + 511) // 512

    ndev = NCORES if collectives else 1
    nc = bacc.Bacc(
        "TRN2", target_bir_lowering=False, debug=False, num_devices=ndev
    )
    core_ids = list(range(NCORES))

    def EIN(name, shape, dt):
        return nc.dram_tensor(name, list(shape), dt, kind="ExternalInput")

    eaT = EIN("eaT", (ET, 33, P), BF16)
    onehot = EIN("onehot", (ET, P, P), BF16)
    idxw = EIN("idxw", (P, SLOTS // 16), I16)
    xT = EIN("xT", (XF, NCOLS), BF16)
    xTf = EIN("xTf", (XF, NP_), BF16)
    batchoh = EIN("batchoh", (C, P, NG), BF16)
    w1aug = [EIN(f"w1aug_{l}", (33, HID), BF16) for l in (1, 2)]
    w2v = [EIN(f"w2v_{l}", (P, JBLK, HID), BF16) for l in (1, 2)]
    root = [EIN(f"root_{l}", (HID, HID), BF16) for l in (1, 2)]
    rootb = [EIN(f"rootb_{l}", (1, HID), BF16) for l in (1, 2)]
    nfc_w = EIN("nfc_w", (XF, HID), BF16)
    nfc_b = EIN("nfc_b", (1, HID), BF16)
    fc1_w = EIN("fc1_w", (HID, 32), BF16)
    fc1_b = EIN("fc1_b", (1, 32), BF16)
    fc2_w = EIN("fc2_w", (32, 1), BF16)
    fc2_b = EIN("fc2_b", (1, 1), BF16)
    identity = EIN("identity", (P, P), BF16)
    ones_row = EIN("ones_row", (1, 512), BF16)
    zrow = EIN("zrow", (16, P), BF16)

    outv = nc.dram_tensor("outv", [1, NG], FP32, kind="ExternalOutput")

    hbuf = [nc.dram_tensor(f"hbuf{l}", [NPH, P], BF16, addr_space="Shared")
            for l in (1, 2)]
    hown = nc.dram_tensor("hown2", [NCOLS, P], BF16)
    gdram = nc.dram_tensor("gdram", [NCOLS, JEXT], BF16)
    ccin = nc.dram_tensor("ccin", [HID, NG], FP32)
    ccout = nc.dram_tensor("ccout", [HID, NG], FP32, addr_space="Shared")

    with tile.TileContext(nc) as tc:
        with (
            tc.tile_pool(name="const", bufs=1) as cpool,
            tc.tile_pool(name="resid", bufs=1) as rpool,
            tc.tile_pool(name="stream", bufs=2) as spool,
            tc.tile_pool(name="small", bufs=4) as smpool,
            tc.tile_pool(name="ps", bufs=8, space="PSUM") as ps,
        ):
            _lc = [0]
            def load_const(t, dt=BF16):
                sb = cpool.tile(list(t.shape), dt, tag=t.name)
                eng = nc.sync if _lc[0] % 2 == 0 else nc.scalar
                _lc[0] += 1
                eng.dma_start(out=sb[:], in_=t[:])
                return sb

            sb_w1 = [load_const(w1aug[i]) for i in range(2)]
            sb_w2v = [load_const(w2v[i]) for i in range(2)]
            sb_root = [load_const(root[i]) for i in range(2)]
            sb_rootb = [load_const(rootb[i]) for i in range(2)]
            sb_nfc_w = load_const(nfc_w)
            sb_nfc_b = load_const(nfc_b)
            sb_fc1_w = load_const(fc1_w)
            sb_fc1_b = load_const(fc1_b)
            sb_fc2_w = load_const(fc2_w)
            sb_fc2_b = load_const(fc2_b)
            sb_ident = load_const(identity)
            sb_ones = load_const(ones_row)
            sb_idx = load_const(idxw, dt=I16)


            def _dummy_out():
                osb0 = smpool.tile([1, NG], FP32, tag="osb0")
                nc.gpsimd.memset(osb0[:], 0.0)
                nc.sync.dma_start(out=outv[:], in_=osb0[:])

            zr_sb = cpool.tile([16, P], BF16, tag="zrsb")
            nc.sync.dma_start(out=zr_sb[:], in_=zrow[:])
            for l in range(2):
                nc.sync.dma_start(out=hbuf[l][NP_:NPH, :], in_=zr_sb[:])

            # ---- startup: h1 (own + all-nodes) and z for both layers,
            # round-robin interleaved so PSUM slots and engines overlap ----
            h1T = rpool.tile([HID, NCOLS], BF16, tag="h1T")
            NPB = NP_ // 512
            XGRP = 4  # 512-blocks per xTf DMA
            xtf_box = [None]

            def emit_h1own(g):
                c0, c1 = g * 512, min((g + 1) * 512, NCOLS)
                w = c1 - c0
                xt = smpool.tile([XF, 512], BF16, tag="xt", bufs=2)
                nc.sync.dma_start(out=xt[:, :w], in_=xT[:, c0:c1])
                pt = ps.tile([P, 512], FP32, tag="b512")
                agg = pt[:HID, :w]
                nc.tensor.matmul(agg, sb_nfc_w[:], xt[:, :w], start=True, stop=False)
                nc.tensor.matmul(agg, sb_nfc_b[:], sb_ones[:1, :w],
                                 start=False, stop=True)
                _leaky(nc, smpool, h1T[:, c0:c1], agg, "h1")

            def emit_h1full(g):
                c0 = g * 512
                if g % XGRP == 0:
                    gw = min(XGRP * 512, NP_ - c0)
                    xtf_box[0] = smpool.tile([XF, XGRP * 512], BF16, tag="xtf",
                                             bufs=2, name="xtf")
                    nc.scalar.dma_start(
                        out=xtf_box[0][:, :gw], in_=xTf[:, c0 : c0 + gw])
                xt = xtf_box[0][:, (g % XGRP) * 512 : (g % XGRP + 1) * 512]
                pt = ps.tile([P, 512], FP32, tag="b512")
                agg = pt[:HID, :]
                nc.tensor.matmul(agg, sb_nfc_w[:], xt, start=True, stop=False)
                nc.tensor.matmul(agg, sb_nfc_b[:], sb_ones[:1, :512],
                                 start=False, stop=True)
                htmp = smpool.tile([HID, 512], BF16, tag="hful", bufs=3)
                _leaky(nc, smpool, htmp[:], agg, "hf")
                tp = ps.tile([P, 1024], BF16, tag="b512")
                for q in range(4):
                    nc.tensor.transpose(
                        tp[:, q * HID : (q + 1) * HID],
                        htmp[:, q * P : (q + 1) * P], sb_ident[:HID, :HID],
                    )
                rows = smpool.tile([P, 4 * HID], BF16, tag="rows4", bufs=3)
                nc.scalar.activation(rows[:], tp[:, : 4 * HID], AF.Copy)
                nc.sync.dma_start(
                    out=hbuf[0][c0 : c0 + 512, :HID].rearrange(
                        "(q p) h -> p q h", q=4
                    ),
                    in_=rows[:].rearrange("p (q h) -> p q h", q=4),
                )

            zsb_l = []
            EPARTS = 2 if C <= 10 else 4
            EH = (ET + EPARTS - 1) // EPARTS
            et_tiles = {}

            def emit_zgroup(l, half, tg):
                zsb = zsb_l[l]
                h0, h1 = half * EH, min((half + 1) * EH, ET)  # half = part idx
                if (l, half) not in et_tiles:
                    et = smpool.tile([33, EH, P], BF16, tag="eat", bufs=2)
                    nc.sync.dma_start(
                        out=et[:, : h1 - h0, :],
                        in_=eaT[h0:h1, :, :].rearrange("t f j -> f t j"),
                    )
                    et_tiles[(l, half)] = et
                et = et_tiles[(l, half)]
                t0, t1 = h0 + tg * 8, min(h0 + tg * 8 + 8, h1)
                zp = ps.tile([P, 512], FP32, tag="b512")
                for t in range(t0, t1):
                    nc.tensor.matmul(
                        zp[:, (t - t0) * HID : (t - t0 + 1) * HID],
                        et[:, t - h0, :], sb_w1[l][:], start=True, stop=True,
                    )
                nc.scalar.activation(
                    zsb[:, t0:t1, :].rearrange("p a b -> p (a b)"),
                    zp[:, : (t1 - t0) * HID], AF.Relu,
                )

            if phases > 0:
                zsb_l = [rpool.tile([P, ET, HID], BF16, tag=f"z{l}",
                                    name=f"zsb{l}")
                         for l in range(2)]
                zjobs = [(0, half, tg)
                         for half in range(EPARTS)
                         for tg in range((min((half + 1) * EH, ET)
                                          - half * EH + 7) // 8)]
                zjobs_l1 = [(1, half, tg)
                            for half in range(EPARTS)
                            for tg in range((min((half + 1) * EH, ET)
                                             - half * EH + 7) // 8)]
            else:
                zjobs = []
                zjobs_l1 = []
            zi = 0
            for g in range(NPB):
                emit_h1full(g)
                if zi < len(zjobs):
                    emit_zgroup(*zjobs[zi])
                    zi += 1
            while zi < len(zjobs):
                emit_zgroup(*zjobs[zi])
                zi += 1
            for g in range(G512):
                emit_h1own(g)

            if phases <= 0:
                _dummy_out()
            hprevT = h1T
            for l in range(2) if phases > 0 else []:
                zsb = zsb_l[l]
                if phases <= 1 + 10 * l:
                    _dummy_out()
                    break
                # ---- gather h rows (split for pipelining) ----
                hs = rpool.tile([P, ET, P], BF16, tag="hs",
                                bufs=2 if C <= 10 else 1)
                for t0s, t1s in ((0, ET // 2), (ET // 2, ET)):
                    ntile = t1s - t0s
                    nc.gpsimd.dma_gather(
                        out_ap=hs[:, t0s:t1s, :], in_ap=hbuf[l][:, :],
                        idxs_ap=sb_idx[:, t0s * 8 : t1s * 8],
                        num_idxs=ntile * P, num_idxs_reg=ntile * P,
                        elem_size=P, single_packet=False,
                    )

                if phases <= 2 + 10 * l:
                    _dummy_out()
                    break

                hnextT = rpool.tile([HID, NCOLS], BF16, tag=f"hn_{l}")


                # ---- per chunk: u outer products + scatter matmuls ----
                for ci in range(C):
                    gq = [ps.tile([P, 512], FP32, tag="b512", name=f"gq{_q}")
                          for _q in range(8)]
                    oh = smpool.tile([P, TPC, P], BF16, tag="oh", bufs=3)
                    nc.sync.dma_start(
                        out=oh[:],
                        in_=onehot[ci * TPC : (ci + 1) * TPC, :, :].rearrange(
                            "t p j -> p t j"
                        ),
                    )
                    for tt in range(TPC):
                        t = ci * TPC + tt
                        zin = zsb[:, t, :, None].broadcast_to([P, HID, HID])
                        hin = hs[:, t, None, :HID].broadcast_to([P, HID, HID])
                        u = spool.tile([P, HID * HID], BF16, tag="u", bufs=3)
                        uview = u[:].rearrange("p (a b) -> p a b", a=HID)
                        route = ROUTE_PAT[ci % len(ROUTE_PAT)][tt]
                        if route in ("A", "G"):
                            zrep = spool.tile([P, HID * HID], BF16, tag="zrep", bufs=3)
                            zview = zrep[:].rearrange("p (a b) -> p a b", a=HID)
                            if route == "A":
                                nc.scalar.activation(zview, zin, AF.Copy)
                            else:
                                nc.gpsimd.tensor_copy(zview, zin)
                            nc.vector.tensor_tensor(
                                out=uview, in0=zview, in1=hin, op=OP.mult
                            )
                        elif route == "D":
                            # direct 1x-mode TT on DVE (broadcast APs)
                            nc.vector.tensor_tensor(
                                out=uview, in0=zin, in1=hin, op=OP.mult
                            )
                        else:  # "P": direct multiply on GPSIMD
                            nc.gpsimd.scalar_tensor_tensor(
                                out=uview, in0=zin, scalar=1.0, in1=hin,
                                op0=OP.mult, op1=OP.mult,
                            )
                        for q in range(8):
                            nc.tensor.matmul(
                                gq[q][:], oh[:, tt, :], u[:, q * 512 : (q + 1) * 512],
                                start=(tt == 0), stop=(tt == TPC - 1),
                            )
                    # b2 pass: H_agg chunk = segsum of h_src rows
                    gb = ps.tile([P, 512], FP32, tag="b512")
                    for t2 in range(TPC):
                        nc.tensor.matmul(
                            gb[:, :HID], oh[:, t2, :], hs[:, ci * TPC + t2, :HID],
                            start=(t2 == 0), stop=(t2 == TPC - 1),
                        )
                    gs = smpool.tile([P, JEXT], BF16, tag="gs", bufs=2)
                    for q in range(8):
                        dst = gs[:, q * 512 : (q + 1) * 512]
                        if q % 2 == 0:
                            nc.scalar.activation(dst, gq[q][:], AF.Copy)
                        else:
                            nc.vector.tensor_copy(dst, gq[q][:])
                    nc.scalar.activation(gs[:, 4096 : 4096 + HID], gb[:, :HID], AF.Copy)
                    nc.gpsimd.memset(gs[:, 4096 + HID : JEXT], 0.0)
                    nc.sync.dma_start(
                        out=gdram[ci * P : (ci + 1) * P, :], in_=gs[:]
                    )

                if phases <= 3 + 10 * l:
                    _dummy_out()
                    break
                if phases <= 4 + 10 * l:
                    _dummy_out()
                    break
                # ---- G^T via DMA transpose, streamed per j-block ----
                aggs = [ps.tile([P, 512], FP32, tag="b512", name=f"agg{l}_{_g}")
                        for _g in range(G512)]
                for b in range(JBLK):
                    gtb = smpool.tile([P, NCOLS], BF16, tag="gtb", bufs=4)
                    nc.sync.dma_start_transpose(
                        gtb[:], gdram[:, b * P : (b + 1) * P]
                    )
                    for g in range(G512):
                        c0, c1 = g * 512, min((g + 1) * 512, NCOLS)
                        nc.tensor.matmul(
                            aggs[g][:HID, : c1 - c0], sb_w2v[l][:, b, :],
                            gtb[:, c0:c1], start=(b == 0), stop=False,
                        )
                    if l == 0 and b % 3 == 0 and zjobs_l1:
                        emit_zgroup(*zjobs_l1.pop(0))
                while l == 0 and zjobs_l1:
                    emit_zgroup(*zjobs_l1.pop(0))
                for g in range(G512):
                    c0, c1 = g * 512, min((g + 1) * 512, NCOLS)
                    w = c1 - c0
                    agg = aggs[g][:HID, :w]
                    nc.tensor.matmul(agg, sb_root[l][:], hprevT[:, c0:c1],
                                     start=False, stop=False)
                    nc.tensor.matmul(agg, sb_rootb[l][:], sb_ones[:1, :w],
                                     start=False, stop=True)
                    _leaky(nc, smpool, hnextT[:, c0:c1], agg, "hn")
                    if l == 0:
                        for ci2 in range(c0 // P, (c1 + P - 1) // P):
                            tp = ps.tile([P, 1024], BF16, tag="b512")
                            nc.tensor.transpose(
                                tp[:, :HID], hnextT[:, ci2 * P : (ci2 + 1) * P],
                                sb_ident[:HID, :HID],
                            )
                            rows = smpool.tile([P, HID], BF16, tag="rows")
                            nc.scalar.activation(rows[:], tp[:, :HID], AF.Copy)
                            nc.sync.dma_start(
                                out=hown[ci2 * P : (ci2 + 1) * P, :HID],
                                in_=rows[:],
                            )

                if phases <= 5 + 10 * l:
                    _dummy_out()
                    break
                # ---- tail ----
                if l == 0:
                    if collectives:
                        nc.gpsimd.collective_compute(
                            "AllGather", OP.bypass, replica_groups=[core_ids],
                            ins=[hown[:]], outs=[hbuf[1][0:NP_, :]],
                        )
                    else:
                        nc.sync.dma_start(out=hbuf[1][0:NCOLS, :], in_=hown[:])
                    hprevT = hnextT
                else:
                    # pooling: hg^T[f, g] = sum_n hx2[n, f] batchoh[n, g]
                    hg_ps = ps.tile([P, 512], FP32, tag="b512")
                    hg = hg_ps[:HID, :NG]
                    boall = smpool.tile([P, C, NG], BF16, tag="bo")
                    nc.sync.dma_start(
                        out=boall[:],
                        in_=batchoh[:, :, :].rearrange("c p g -> p c g"),
                    )
                    for ci in range(C):
                        tp = ps.tile([P, 1024], BF16, tag="b512")
                        nc.tensor.transpose(
                            tp[:, :HID], hnextT[:, ci * P : (ci + 1) * P],
                            sb_ident[:HID, :HID],
                        )
                        rows = smpool.tile([P, HID], BF16, tag="rows")
                        nc.scalar.activation(rows[:], tp[:, :HID], AF.Copy)
                        nc.tensor.matmul(hg, rows[:], boall[:, ci, :],
                                         start=(ci == 0), stop=(ci == C - 1))
                    hg_sb = smpool.tile([HID, NG], FP32, tag="hgsb")
                    nc.vector.tensor_copy(hg_sb[:], hg)
                    nc.sync.dma_start(out=ccin[:], in_=hg_sb[:])
                    if collectives:
                        nc.gpsimd.collective_compute(
                            "AllReduce", OP.add, replica_groups=[core_ids],
                            ins=[ccin[:]], outs=[ccout[:]],
                        )
                    else:
                        nc.sync.dma_start(out=ccout[:], in_=ccin[:])
                    hgT = smpool.tile([HID, NG], FP32, tag="hgT")
                    nc.sync.dma_start(out=hgT[:], in_=ccout[:])
                    hgTb = smpool.tile([HID, NG], BF16, tag="hgTb")
                    nc.scalar.activation(hgTb[:], hgT[:], AF.Copy)
                    z1p = ps.tile([P, 512], FP32, tag="b512")
                    z1 = z1p[:32, :NG]
                    nc.tensor.matmul(z1, sb_fc1_w[:], hgTb[:], start=True, stop=False)
                    nc.tensor.matmul(z1, sb_fc1_b[:], sb_ones[:1, :NG],
                                     start=False, stop=True)
                    z1sb = smpool.tile([32, NG], BF16, tag="z1sb")
                    _leaky(nc, smpool, z1sb[:], z1, "z1")
                    op_ = ps.tile([P, 512], FP32, tag="b512")
                    o1 = op_[:1, :NG]
                    nc.tensor.matmul(o1, sb_fc2_w[:], z1sb[:], start=True, stop=False)
                    nc.tensor.matmul(o1, sb_fc2_b[:], sb_ones[:1, :NG],
                                     start=False, stop=True)
                    osb = smpool.tile([1, NG], FP32, tag="osb")
                    nc.vector.tensor_copy(osb[:], o1)
                    nc.sync.dma_start(out=outv[:], in_=osb[:])

    nc.compile()
    return nc


# ----------------------------------------------------------------------------
# Entry point
# ----------------------------------------------------------------------------
_BUILD_CACHE = {}


def build_in_maps(inputs):
    edge_index = np.asarray(inputs["edge_index"])
    batch = np.asarray(inputs["batch"])
    plan = _plan(edge_index, batch)
    C = plan["C"]
    shared = _shared_inputs(inputs)
    shared["xTf"] = _full_xT(plan, inputs["x"])
    in_maps = []
    for c in range(NCORES):
        m = dict(shared)
        m.update(_per_core_inputs(plan, c, inputs["edge_attr"], inputs["x"]))
        in_maps.append({k: np.ascontiguousarray(v) for k, v in m.items()})
    return C, in_maps


def kernel(**inputs) -> np.ndarray:
    C, in_maps = build_in_maps(inputs)
    if C not in _BUILD_CACHE:
        _BUILD_CACHE[C] = _build(C)
    nc = _BUILD_CACHE[C]
    res = run_bass_kernel_spmd(nc, in_maps, list(range(NCORES)))
    out = np.asarray(res.results[0]["outv"], dtype=np.float32)
    return out.reshape(NG, 1)


if __name__ == "__main__":
    import reference

    inp = {k: np.asarray(v) for k, v in reference.setup_inputs().items()}
    got = kernel(**inp)
    exp = np.asarray(reference.reference(**inp))
    num = np.linalg.norm(got - exp)
    den = np.linalg.norm(exp) + 1e-30
    print("rel l2 error:", num / den)
    print("max abs diff:", np.abs(got - exp).max(), "absmax:", np.abs(exp).max())

